# revision 20
# baseline (speedup 1.0000x reference)
"""AttentionLSTM Trainium2 kernel (8-core data-parallel, Bass/Tile).

Reference computation (per step t, batch N=256, H=D=1024):
    scores = einsum('nh,nhp->np', h, A_flat) / sqrt(H)
    w      = softmax(scores)                      # over 16 spatial positions
    attn   = einsum('nhp,np->nh', A_flat, w)
    a      = x_t @ Wx + h @ Wh + attn @ Wattn + b
    i,f,o,g = split(a, 4); c = sig(f)*c + sig(i)*tanh(g); h = sig(o)*tanh(c)

Mapping:
  * Data-parallel: batch 256 -> 8 cores x 32.
  * Inputs ship in natural layout (bf16); all layout transformation
    happens on device (PE transposes).  Weights ship K-sharded (each
    core uploads a distinct 128-row chunk of Wx/Wh/Wattn) and are
    reassembled on device with a NeuronLink AllGather, cutting host->
    device traffic ~8x for the replicated weights.
  * xproj = x @ Wx + b hoisted out of the scan (bias folded in as a
    ones-row K-chunk), staged through DRAM scratch in bf16.
  * attn @ Wattn re-associated: G[(n,p),:] = A[n,:,p] @ Wattn once,
    per step the attention contribution is w_blockdiag.T @ G.
  * scores via a cross-product matmul + masked diagonal extract.
  * softmax exp computed as sig(x)/(1-sig(x)) (x <= 0 post max-sub) so
    the recurrence stays in one ACT table set (no per-step table loads).
  * All matmul operands bf16; accumulation/state f32; output bf16.
"""
import sys

sys.path.insert(0, "/opt/trn_rl_repo")

import numpy as np
import ml_dtypes

import concourse.bacc as bacc
import concourse.bass as bass
import concourse.tile as tile
from concourse import mybir

BF16 = ml_dtypes.bfloat16
F32 = mybir.dt.float32
BF = mybir.dt.bfloat16
AF = mybir.ActivationFunctionType
AX = mybir.AxisListType
ALU = mybir.AluOpType

N_CORES = 8
N, T, D, H = 256, 64, 1024, 1024
NL = N // N_CORES            # 32 batch per core
HC = H // 128                # 8 K-chunks of the hidden dim
NB = 8                       # 512-wide gate column chunks
P16N = 16                    # attention positions
SCALE = 1.0 / float(np.sqrt(H))


def _ap(t, off, dims):
    """AP over tile/AP `t` at element offset `off` with dims [[stride, n], ...]."""
    return bass.AP(tensor=t.tensor, offset=t.offset + off, ap=dims)


def _build_program(t_steps=T):
    nc = bacc.Bacc("TRN2", target_bir_lowering=False, debug=False,
                   num_devices=N_CORES)

    # ---- DRAM I/O (per core; natural layouts) ----
    # x rows are n-major: row = n*T + t
    d_x = nc.dram_tensor("x", (NL * t_steps, D), BF, kind="ExternalInput").ap()
    d_a = nc.dram_tensor("A", (NL, H * P16N), BF, kind="ExternalInput").ap()
    # this core's K-chunk of [Wx | Wh | Wattn] (natural gate column order)
    d_wsh = nc.dram_tensor("Wsh", (128, 3, 4096), BF, kind="ExternalInput").ap()
    d_bias = nc.dram_tensor("bias", (1, 4096), BF, kind="ExternalInput").ap()
    d_mdiag = nc.dram_tensor("mask_diag", (NL, NL * P16N), F32, kind="ExternalInput").ap()
    d_p16 = nc.dram_tensor("P16", (16, 128), BF, kind="ExternalInput").ap()
    d_mbd = nc.dram_tensor("mask_bd", (128, 4, NL), F32, kind="ExternalInput").ap()
    d_ident = nc.dram_tensor("ident", (128, 128), BF, kind="ExternalInput").ap()
    d_red4 = nc.dram_tensor("red4", (128, 32), BF, kind="ExternalInput").ap()
    d_hs = nc.dram_tensor("hs", (NL, t_steps, H), BF, kind="ExternalOutput").ap()

    MT = (NL * t_steps) // 128   # xproj M-tiles (n-major rows)

    with tile.TileContext(nc) as tc:
        with tc.tile_pool(name="resident", bufs=1) as resident, \
             tc.tile_pool(name="dram", bufs=1, space="DRAM") as dram:
            xp_d = dram.tile([NL * t_steps, NB, 512], BF)

            # ---------- weight AllGather (K-sharded upload) ----------
            # one combined gather: per-op overhead is high and effective
            # bandwidth improves with transfer size
            wbounce = dram.tile([128, 3 * 4096], BF, name="wbounce")
            wgfull = dram.tile([H, 3 * 4096], BF, name="wgfull")
            nc.gpsimd.dma_start(out=wbounce, in_=d_wsh)
            nc.gpsimd.collective_compute(
                "AllGather",
                ALU.bypass,
                replica_groups=[list(range(N_CORES))],
                ins=[wbounce.opt()],
                outs=[wgfull.opt()],
            )
            WROW = 3 * 4096   # gathered row stride; weight w at col w*4096

            # ---------- resident tiles ----------
            ident = resident.tile([128, 128], BF)
            nc.sync.dma_start(out=ident, in_=d_ident)
            mdiag = resident.tile([NL, NL * P16N], F32)
            nc.sync.dma_start(out=mdiag, in_=d_mdiag)
            p16 = resident.tile([16, 128], BF)
            nc.sync.dma_start(out=p16, in_=d_p16)
            mbd = resident.tile([128, 4, NL], F32)
            nc.sync.dma_start(out=mbd, in_=d_mbd)
            w_pad = resident.tile([NL, 32], BF)
            nc.vector.memset(w_pad, 0.0)
            red4 = resident.tile([128, 32], BF)
            nc.sync.dma_start(out=red4, in_=d_red4)

            a_r = resident.tile([128, HC, NL * P16N], BF)
            g_r = resident.tile([128, 4, 4096], BF)
            bias_aug = resident.tile([128, 4096], BF)
            nc.vector.memset(bias_aug, 0.0)
            nc.sync.dma_start(out=bias_aug[0:1, :], in_=d_bias)

            with tc.tile_pool(name="state", bufs=2) as state:

                # ---------- phase A: A-derived tensors ----------
                with tc.tile_pool(name="apre", bufs=1) as apre, \
                     tc.tile_pool(name="tps", bufs=4, space="PSUM") as tps:
                    a_sb = apre.tile([NL, H * P16N], BF, tag="a_sb")
                    nc.sync.dma_start(out=a_sb, in_=d_a)
                    arp = a_sb.ap[0][0]   # partition stride of a_sb
                    for hc in range(HC):
                        for p in range(P16N):
                            ps = tps.tile([128, NL], BF, tag="tp",
                                          name=f"tpa_{hc}_{p}")
                            nc.tensor.transpose(
                                ps,
                                _ap(a_sb, hc * 2048 + p, [[arp, NL], [16, 128]]),
                                ident[0:NL, 0:NL])
                            # a_r[:, hc, n*16+p] = A[n, hc*128+:, p]
                            nc.scalar.copy(
                                _ap(a_r, hc * 512 + p,
                                    [[a_r.ap[0][0], 128], [16, NL]]),
                                ps)

                    # c0 = mean_p A  (n-major), state f32
                    c_cur = state.tile([NL, H], F32, tag="c")
                    csum = apre.tile([NL, H], F32, tag="csum")
                    nc.vector.reduce_sum(
                        csum, _ap(a_sb, 0, [[arp, NL], [16, H], [1, 16]]),
                        axis=AX.X)
                    nc.vector.tensor_scalar_mul(c_cur, csum, 1.0 / 16.0)

                    # hT0 from a_r: mean over p, per K-chunk
                    hT_cur = state.tile([128, HC, NL], BF, tag="hT")
                    for hc in range(HC):
                        rsum = apre.tile([128, NL], F32, tag="rsum",
                                         name=f"rsum_{hc}")
                        nc.vector.reduce_sum(
                            rsum,
                            _ap(a_r, hc * 512,
                                [[a_r.ap[0][0], 128], [16, NL], [1, 16]]),
                            axis=AX.X)
                        nc.scalar.activation(hT_cur[:, hc, :], rsum, AF.Copy,
                                             scale=1.0 / 16.0)

                # ---------- phases B-D share xt_full ----------
                xtp_cm = tc.tile_pool(name="xtp", bufs=1)
                xtp = xtp_cm.__enter__()
                xt_full = xtp.tile([128, 9, NL * t_steps], BF)

                # ---------- phase B: x^T tiles (PE transposes) ----------
                with tc.tile_pool(name="xload", bufs=3) as xload, \
                     tc.tile_pool(name="tpsb", bufs=4, space="PSUM") as tps:
                    for mt in range(MT):
                        # tile rows are t-major: row = t*NL + n, t = 4*mt + tl
                        x_sb = xload.tile([128, D], BF, tag="x_sb",
                                          name=f"xsb_{mt}")
                        for tl in range(4):
                            nc.sync.dma_start(
                                out=x_sb[32 * tl:32 * (tl + 1), :],
                                in_=_ap(d_x, (4 * mt + tl) * D,
                                        [[t_steps * D, NL], [1, D]]))
                        for kc in range(8):
                            ps = tps.tile([128, 128], BF, tag="tp",
                                          name=f"tpx_{mt}_{kc}")
                            nc.tensor.transpose(
                                ps, x_sb[:, kc * 128:(kc + 1) * 128], ident)
                            nc.scalar.copy(
                                xt_full[:, kc, mt * 128:(mt + 1) * 128], ps)
                nc.vector.memset(xt_full[:, 8, :], 0.0)
                nc.vector.memset(xt_full[0:1, 8, :], 1.0)

                # ---------- phase C: G = A^T @ Wattn ----------
                with tc.tile_pool(name="gpre", bufs=3) as gpre, \
                     tc.tile_pool(name="gps", bufs=4, space="PSUM") as gps:
                    for nb in range(NB):
                        psum_g = [gps.tile([128, 512], F32, tag="gq",
                                           name=f"psum_g{nb}_{q}")
                                  for q in range(4)]
                        for kc in range(HC):
                            wa_t = gpre.tile([128, 512], BF, tag="wa")
                            nc.sync.dma_start(
                                out=wa_t,
                                in_=_ap(wgfull,
                                        kc * 128 * WROW + 2 * 4096 + nb * 512,
                                        [[WROW, 128], [1, 512]]))
                            for q in range(4):
                                nc.tensor.matmul(
                                    psum_g[q],
                                    lhsT=a_r[:, kc, 128 * q:128 * (q + 1)],
                                    rhs=wa_t,
                                    start=(kc == 0), stop=(kc == HC - 1))
                        for q in range(4):
                            nc.scalar.copy(g_r[:, q, nb * 512:(nb + 1) * 512],
                                           psum_g[q])

                # ---------- phase D: xproj = x @ Wx + b ----------
                with tc.tile_pool(name="xwc", bufs=2) as xwc, \
                     tc.tile_pool(name="xout", bufs=4) as xout, \
                     tc.tile_pool(name="xps", bufs=4, space="PSUM") as xps:
                    for nb in range(NB):
                        wxc = xwc.tile([128, 8, 512], BF, tag="wxc")
                        nc.sync.dma_start(
                            out=wxc,
                            in_=_ap(wgfull, nb * 512,
                                    [[WROW, 128], [128 * WROW, 8], [1, 512]]))
                        for mt in range(MT):
                            psum_xp = xps.tile([128, 512], F32, tag="xp")
                            for kc in range(9):
                                rhs = (wxc[:, kc, :] if kc < 8
                                       else bias_aug[:, nb * 512:(nb + 1) * 512])
                                nc.tensor.matmul(
                                    psum_xp,
                                    lhsT=xt_full[:, kc, mt * 128:(mt + 1) * 128],
                                    rhs=rhs,
                                    start=(kc == 0), stop=(kc == 8))
                            xo = xout.tile([128, 512], BF, tag="xo")
                            nc.scalar.copy(xo, psum_xp)
                            nc.sync.dma_start(
                                out=xp_d[mt * 128:(mt + 1) * 128, nb, :], in_=xo)

                xtp_cm.__exit__(None, None, None)

                # ---------- phase E: recurrence ----------
                wh_s = resident.tile([128, HC, 4096], BF)
                nc.sync.dma_start(
                    out=wh_s,
                    in_=_ap(wgfull, 1 * 4096,
                            [[WROW, 128], [128 * WROW, 8], [1, 4096]]))

                with tc.tile_pool(name="xpin", bufs=10) as xpin, \
                     tc.tile_pool(name="work", bufs=2) as work, \
                     tc.tile_pool(name="acts", bufs=2) as acts, \
                     tc.tile_pool(name="hout", bufs=3) as hout, \
                     tc.tile_pool(name="ps_we", bufs=1, space="PSUM") as ps_we, \
                     tc.tile_pool(name="ps_w", bufs=4, space="PSUM") as ps_w, \
                     tc.tile_pool(name="ps_pre", bufs=3, space="PSUM") as ps_pre:

                    # Column-group j of a col-tiled matmul takes K-chunks
                    # {j, 4+j}: round-0 work needs only the FIRST half of
                    # h^T, so all round-0 PE work is emitted before any
                    # round-1 work (the PE queue is in-order) and the PE
                    # stays busy through the previous step's cell-update
                    # tail -- the HAM throttle never re-engages.  h^T
                    # transposes run on the PE at the end of the step for
                    # the same reason.  Odd chunks are ordered so the
                    # half-1 cell chain's inputs (i=1, g=7, f=3) are ready
                    # earliest.
                    EVEN, ODD = [0, 2, 4, 6], [1, 7, 3, 5]

                    def _hpart_round(pw, r, nbs):
                        for j in range(4):
                            kc = 4 * r + j
                            nc.tensor.matmul(
                                pw[32 * j:32 * (j + 1), :],
                                lhsT=hT_cur[:, kc, :],
                                rhs=wh_s[:, kc, nbs],
                                start=(r == 0), stop=False,
                                tile_position=(0, 32 * j))

                    pending = [None]   # previous step's half-1 h^T transposes

                    for t in range(t_steps):
                        xp_t = []
                        for nb in range(NB):
                            xpc = xpin.tile([NL, 512], BF, tag="xp_t",
                                            name=f"xp_{t}_{nb}")
                            # xp_d rows are t-major: row = t*NL + n
                            nc.sync.dma_start(
                                out=xpc, in_=xp_d[t * NL:(t + 1) * NL, nb, :])
                            xp_t.append(xpc)

                        # --- PE round 0 (needs only half-0 of h^T) ---
                        psum_scw = ps_w.tile([128, NL * P16N], F32, tag="w",
                                             name=f"scw_{t}")
                        for j in range(4):
                            nc.tensor.matmul(
                                psum_scw[32 * j:32 * (j + 1), :],
                                lhsT=hT_cur[:, j, :], rhs=a_r[:, j, :],
                                start=True, stop=False,
                                tile_position=(0, 32 * j))
                        psum_ws = {}
                        for nb in EVEN:
                            pw = ps_w.tile([128, 512], F32, tag="w",
                                           name=f"gw_{t}_{nb}")
                            psum_ws[nb] = pw
                            _hpart_round(pw, 0, slice(nb * 512, (nb + 1) * 512))

                        # previous step's half-1 h^T transposes: deferred
                        # to here so they don't block this step's round-0
                        # matmuls in the in-order PE queue
                        if pending[0] is not None:
                            pending[0]()
                            pending[0] = None

                        # --- PE round 1 (needs half-1 of h^T) ---
                        for j in range(4):
                            nc.tensor.matmul(
                                psum_scw[32 * j:32 * (j + 1), :],
                                lhsT=hT_cur[:, 4 + j, :], rhs=a_r[:, 4 + j, :],
                                start=False, stop=True,
                                tile_position=(0, 32 * j))
                        sb_scw = work.tile([128, NL * P16N], BF, tag="scw",
                                           name=f"sbscw_{t}")
                        nc.vector.tensor_copy(sb_scw, psum_scw)
                        psum_sc = ps_pre.tile([NL, NL * P16N], F32, tag="pre",
                                              bufs=2, name=f"psc_{t}")
                        nc.tensor.matmul(psum_sc, lhsT=red4, rhs=sb_scw,
                                         start=True, stop=True)
                        for nb in EVEN:
                            _hpart_round(psum_ws[nb], 1,
                                         slice(nb * 512, (nb + 1) * 512))

                        # --- softmax over the 16 positions (DVE/ACT;
                        #     overlaps the round-1 matmuls above) ---
                        masked = work.tile([NL, NL * P16N], F32, tag="masked")
                        nc.vector.tensor_mul(masked, psum_sc, mdiag)
                        masked_pm = _ap(masked, 0,
                                        [[masked.ap[0][0], NL], [1, P16N], [P16N, NL]])
                        scores32 = work.tile([NL, P16N], F32, tag="scores")
                        nc.vector.reduce_sum(scores32, masked_pm, axis=AX.X)
                        nmx = work.tile([NL, 1], F32, tag="nmx")
                        nc.vector.reduce_max(nmx, scores32, axis=AX.X, negate=True)
                        nmx_s = work.tile([NL, 1], F32, tag="nmx_s")
                        nc.vector.tensor_scalar_mul(nmx_s, nmx, SCALE)
                        # exp(x) = sig(x)/(1-sig(x)) for x<=0; stays in the
                        # sigmoid ACT table set (no per-step table reload)
                        sg = work.tile([NL, P16N], F32, tag="sg")
                        nc.scalar.activation(sg, scores32, AF.Sigmoid,
                                             bias=nmx_s, scale=SCALE)
                        om = work.tile([NL, P16N], F32, tag="om")
                        nc.vector.tensor_scalar(om, sg, -1.0, 1.0,
                                                ALU.mult, ALU.add)
                        omr = work.tile([NL, P16N], F32, tag="omr")
                        nc.vector.reciprocal(omr, om)
                        e_t = work.tile([NL, P16N], F32, tag="e")
                        nc.vector.tensor_mul(e_t, sg, omr)
                        ssum = work.tile([NL, 1], F32, tag="ssum")
                        nc.vector.reduce_sum(ssum, e_t, axis=AX.X)
                        rr = work.tile([NL, 1], F32, tag="rr")
                        nc.vector.reciprocal(rr, ssum)
                        nc.vector.tensor_scalar_mul(w_pad[:, 0:P16N], e_t, rr)
                        wT_t = work.tile([NL, 32], BF, tag="wT")
                        nc.vector.transpose(out=wT_t, in_=w_pad)
                        psum_we = ps_we.tile([128, NL], F32, tag="we")
                        nc.tensor.matmul(psum_we, lhsT=p16, rhs=wT_t[0:16, :],
                                         start=True, stop=True)
                        w_bdb = work.tile([128, 4, NL], BF, tag="wbd")
                        wexp_b = _ap(psum_we, 0,
                                     [[psum_we.ap[0][0], 128], [0, 4], [1, NL]])
                        nc.vector.tensor_mul(w_bdb, wexp_b, mbd)

                        # --- finish gate chunks: attention round, fold
                        #     column-group partials + xproj (red4 matmul, xp
                        #     added to group 0 during the PSUM->SBUF move),
                        #     then the gate nonlinearity (chunks 6,7 = tanh)
                        gacts = [None] * NB
                        c_nxt = state.tile([NL, H], F32, tag="c")
                        h_bf = hout.tile([NL, H], BF, tag="hbf")
                        hT_nxt = state.tile([128, HC, NL], BF, tag="hT")

                        def _cell_half(j):
                            si, sf, so, tg = (gacts[j], gacts[2 + j],
                                              gacts[4 + j], gacts[6 + j])
                            jsl = slice(j * 512, (j + 1) * 512)
                            t1 = acts.tile([NL, 512], F32, tag="ct", bufs=14,
                                           name=f"t1_{t}_{j}")
                            nc.vector.tensor_mul(t1, sf, c_cur[:, jsl])
                            t2 = acts.tile([NL, 512], F32, tag="ct", bufs=14,
                                           name=f"t2_{t}_{j}")
                            nc.gpsimd.tensor_mul(t2, si, tg)
                            nc.vector.tensor_add(c_nxt[:, jsl], t1, t2)
                            tcn = acts.tile([NL, 512], F32, tag="ct", bufs=14,
                                            name=f"tcn_{t}_{j}")
                            nc.scalar.activation(tcn, c_nxt[:, jsl], AF.Tanh)
                            nc.vector.tensor_mul(h_bf[:, jsl], so, tcn)

                        def _transpose_half(j, hb=None, ht=None):
                            # h^T for this half on the PE (keeps it warm
                            # through the cell-update tail)
                            hb = h_bf if hb is None else hb
                            ht = hT_nxt if ht is None else ht
                            for hc in range(4 * j, 4 * j + 4):
                                tp = ps_we.tile([128, NL], BF, tag="tp",
                                                bufs=1, name=f"tp_{t}_{j}_{hc}")
                                nc.tensor.transpose(
                                    tp, hb[:, hc * 128:(hc + 1) * 128],
                                    ident[0:NL, 0:NL])
                                nc.scalar.copy(ht[:, hc, :], tp)

                        def _finish_nb(nb):
                            nbs = slice(nb * 512, (nb + 1) * 512)
                            pw = psum_ws[nb]
                            for j in range(4):
                                nc.tensor.matmul(
                                    pw[32 * j:32 * (j + 1), :],
                                    lhsT=w_bdb[:, j, :],
                                    rhs=g_r[:, j, nbs],
                                    start=False, stop=True,
                                    tile_position=(0, 32 * j))
                            sb_w = work.tile([128, 512], BF, tag="gw", bufs=4,
                                             name=f"sbw_{t}_{nb}")
                            # xp folds into group 0's partial during the
                            # PSUM->SBUF move; partition-range rule: APs
                            # based at partition 32/64 span <=32/64 rows.
                            nc.vector.tensor_add(sb_w[0:NL, :], pw[0:NL, :],
                                                 xp_t[nb])
                            if nb % 2 == 0:
                                nc.scalar.copy(sb_w[NL:2 * NL, :],
                                               pw[NL:2 * NL, :])
                                nc.scalar.copy(sb_w[2 * NL:128, :],
                                               pw[2 * NL:128, :])
                            else:
                                nc.vector.tensor_copy(sb_w[NL:2 * NL, :],
                                                      pw[NL:2 * NL, :])
                                nc.scalar.copy(sb_w[2 * NL:128, :],
                                               pw[2 * NL:128, :])
                            psum_pre = ps_pre.tile([NL, 512], F32, tag="pre",
                                                   bufs=2, name=f"pre_{t}_{nb}")
                            nc.tensor.matmul(psum_pre, lhsT=red4, rhs=sb_w,
                                             start=True, stop=True)
                            ga = acts.tile([NL, 512], F32, tag="ct", bufs=14,
                                           name=f"ga_{t}_{nb}")
                            nc.scalar.activation(
                                ga, psum_pre,
                                AF.Tanh if nb >= 6 else AF.Sigmoid)
                            gacts[nb] = ga

                        for nb in EVEN:
                            _finish_nb(nb)
                        _cell_half(0)
                        for nb in ODD:
                            pw = ps_w.tile([128, 512], F32, tag="w",
                                           name=f"gw_{t}_{nb}")
                            psum_ws[nb] = pw
                            nbs = slice(nb * 512, (nb + 1) * 512)
                            _hpart_round(pw, 0, nbs)
                            _hpart_round(pw, 1, nbs)
                            _finish_nb(nb)
                        _cell_half(1)
                        _transpose_half(0)
                        pending[0] = (lambda hb=h_bf, ht=hT_nxt:
                                      _transpose_half(1, hb, ht))
                        nc.sync.dma_start(out=d_hs[:, t, :], in_=h_bf)

                        hT_cur = hT_nxt
                        c_cur = c_nxt
                    if pending[0] is not None:
                        pending[0]()

    nc.compile()
    return nc


_PROGRAM_CACHE = {}


def _get_program(t_steps=T):
    if t_steps not in _PROGRAM_CACHE:
        _PROGRAM_CACHE[t_steps] = _build_program(t_steps)
    return _PROGRAM_CACHE[t_steps]


# ---------------------------------------------------------------------------
# Host-side constants (input-independent), cached at module level.
# ---------------------------------------------------------------------------
def _consts():
    mask_diag = np.zeros((NL, NL * P16N), dtype=np.float32)
    for i in range(NL):
        mask_diag[i, i * P16N:(i + 1) * P16N] = 1.0
    p16_m = np.zeros((16, 128), dtype=BF16)
    for j in range(128):
        p16_m[j % 16, j] = 1.0
    mask_bd = np.zeros((128, 4, NL), dtype=np.float32)
    for part in range(128):
        n_lo = part // 16
        for q in range(4):
            mask_bd[part, q, 8 * q + n_lo] = 1.0
    ident = np.eye(128, dtype=BF16)
    red4 = np.tile(np.eye(32, dtype=BF16), (4, 1))
    return {
        "mask_diag": np.tile(mask_diag, (N_CORES, 1)),
        "P16": np.tile(p16_m, (N_CORES, 1)),
        "mask_bd": np.tile(mask_bd, (N_CORES, 1, 1)),
        "ident": np.tile(ident, (N_CORES, 1)),
        "red4": np.tile(red4, (N_CORES, 1)),
    }


_CONSTS = None


def _get_consts():
    global _CONSTS
    if _CONSTS is None:
        _CONSTS = _consts()
    return _CONSTS


# ---------------------------------------------------------------------------
# Cached PJRT runner (jit built once per program).
# ---------------------------------------------------------------------------
_RUNNER_CACHE = {}


def _get_runner(nc):
    key = id(nc)
    if key in _RUNNER_CACHE:
        return _RUNNER_CACHE[key]

    import jax
    import jax.numpy as jnp
    from jax.sharding import Mesh, PartitionSpec, NamedSharding
    from jax.experimental.shard_map import shard_map
    from concourse.bass2jax import (
        install_neuronx_cc_hook, _bass_exec_p, partition_id_tensor)

    install_neuronx_cc_hook()

    partition_name = (nc.partition_id_tensor.name
                      if nc.partition_id_tensor else None)
    in_names, out_names, out_avals = [], [], []
    for alloc in nc.m.functions[0].allocations:
        if not isinstance(alloc, mybir.MemoryLocationSet):
            continue
        name = alloc.memorylocations[0].name
        if alloc.kind == "ExternalInput":
            if name != partition_name:
                in_names.append(name)
        elif alloc.kind == "ExternalOutput":
            out_names.append(name)
            out_avals.append(jax.core.ShapedArray(
                tuple(alloc.tensor_shape), mybir.dt.np(alloc.dtype)))
    n_params = len(in_names)
    all_in_names = list(in_names) + list(out_names)
    if partition_name is not None:
        all_in_names.append(partition_name)

    def _body(*args):
        operands = list(args)
        if partition_name is not None:
            operands.append(partition_id_tensor())
        outs = _bass_exec_p.bind(
            *operands,
            out_avals=tuple(out_avals),
            in_names=tuple(all_in_names),
            out_names=tuple(out_names),
            lowering_input_output_aliases=(),
            sim_require_finite=True,
            sim_require_nnan=True,
            nc=nc,
        )
        return tuple(outs)

    devices = jax.devices()[:N_CORES]
    mesh = Mesh(np.asarray(devices), ("core",))
    n_outs = len(out_names)
    in_specs = (PartitionSpec("core"),) * (n_params + n_outs)
    out_specs = (PartitionSpec("core"),) * n_outs
    donate = tuple(range(n_params, n_params + n_outs))
    sharded = jax.jit(
        shard_map(_body, mesh=mesh, in_specs=in_specs,
                  out_specs=out_specs, check_rep=False),
        donate_argnums=donate, keep_unused=True)
    zero_sh = tuple(NamedSharding(mesh, PartitionSpec("core"))
                    for _ in out_avals)
    make_zeros = jax.jit(
        lambda: tuple(jnp.zeros((N_CORES * av.shape[0], *av.shape[1:]),
                                av.dtype) for av in out_avals),
        out_shardings=zero_sh)

    def run(global_in: dict):
        args = [global_in[name] for name in in_names]
        out_arrs = sharded(*args, *make_zeros())
        return {name: out_arrs[i] for i, name in enumerate(out_names)}

    _RUNNER_CACHE[key] = run
    return run


_PREP_CACHE = {}


def _get_prep():
    if "prep" in _PREP_CACHE:
        return _PREP_CACHE["prep"]
    import jax
    import jax.numpy as jnp
    cpu = jax.devices("cpu")[0]
    bf = jnp.bfloat16

    def _prep(x, A, Wx, Wh, Wattn, b):
        xg = x.reshape(N * T, D).astype(bf)
        ag = A.reshape(N, H * P16N).astype(bf)
        wg = jnp.stack([Wx, Wh, Wattn], axis=1).astype(bf)
        bg = jnp.broadcast_to(b.astype(bf)[None, :], (N_CORES, 4 * H))
        return xg, ag, wg, bg

    def _post(hs):
        return hs.astype(jnp.float32)

    prep = (jax.jit(_prep, device=cpu), jax.jit(_post, device=cpu))
    _PREP_CACHE["prep"] = prep
    return prep


def kernel(**inputs):
    x = np.ascontiguousarray(np.asarray(inputs["x"], dtype=np.float32))
    A = np.ascontiguousarray(np.asarray(inputs["A"], dtype=np.float32))
    Wx = np.asarray(inputs["Wx"], dtype=np.float32)
    Wh = np.asarray(inputs["Wh"], dtype=np.float32)
    Wattn = np.asarray(inputs["Wattn"], dtype=np.float32)
    b = np.asarray(inputs["b"], dtype=np.float32)

    nc = _get_program(T)
    run = _get_runner(nc)
    prep, post = _get_prep()
    xg, ag, wgl, bg = (np.asarray(v) for v in prep(x, A, Wx, Wh, Wattn, b))
    consts = _get_consts()
    global_in = {
        "x": xg, "A": ag, "Wsh": wgl, "bias": bg,
        "mask_diag": consts["mask_diag"], "P16": consts["P16"],
        "mask_bd": consts["mask_bd"], "ident": consts["ident"],
        "red4": consts["red4"],
    }
    outs = run(global_in)
    hs = np.asarray(post(outs["hs"]))          # (256, 64, 1024) f32
    return hs.reshape(N, T, H)


# revision 22
# speedup vs baseline: 1.2256x; 1.2256x over previous
"""AttentionLSTM Trainium2 kernel (8-core data-parallel, Bass/Tile).

Reference computation (per step t, batch N=256, H=D=1024):
    scores = einsum('nh,nhp->np', h, A_flat) / sqrt(H)
    w      = softmax(scores)                      # over 16 spatial positions
    attn   = einsum('nhp,np->nh', A_flat, w)
    a      = x_t @ Wx + h @ Wh + attn @ Wattn + b
    i,f,o,g = split(a, 4); c = sig(f)*c + sig(i)*tanh(g); h = sig(o)*tanh(c)

Mapping:
  * Data-parallel: batch 256 -> 8 cores x 32.
  * Inputs ship in natural layout (bf16); all layout transformation
    happens on device (PE transposes).  Weights ship K-sharded (each
    core uploads a distinct 128-row chunk of Wx/Wh/Wattn) and are
    reassembled on device with a NeuronLink AllGather, cutting host->
    device traffic ~8x for the replicated weights.
  * xproj = x @ Wx + b hoisted out of the scan (bias folded in as a
    ones-row K-chunk), staged through DRAM scratch in bf16.
  * attn @ Wattn re-associated: G[(n,p),:] = A[n,:,p] @ Wattn once,
    per step the attention contribution is w_blockdiag.T @ G.
  * scores via a cross-product matmul + masked diagonal extract.
  * softmax exp computed as sig(x)/(1-sig(x)) (x <= 0 post max-sub) so
    the recurrence stays in one ACT table set (no per-step table loads).
  * All matmul operands bf16; accumulation/state f32; output bf16.
"""
import sys

sys.path.insert(0, "/opt/trn_rl_repo")

import numpy as np
import ml_dtypes

import concourse.bacc as bacc
import concourse.bass as bass
import concourse.tile as tile
from concourse import mybir

BF16 = ml_dtypes.bfloat16
F32 = mybir.dt.float32
BF = mybir.dt.bfloat16
AF = mybir.ActivationFunctionType
AX = mybir.AxisListType
ALU = mybir.AluOpType

N_CORES = 8
N, T, D, H = 256, 64, 1024, 1024
NL = N // N_CORES            # 32 batch per core
HC = H // 128                # 8 K-chunks of the hidden dim
NB = 8                       # 512-wide gate column chunks
P16N = 16                    # attention positions
SCALE = 1.0 / float(np.sqrt(H))


def _ap(t, off, dims):
    """AP over tile/AP `t` at element offset `off` with dims [[stride, n], ...]."""
    return bass.AP(tensor=t.tensor, offset=t.offset + off, ap=dims)


def _build_program(t_steps=T):
    nc = bacc.Bacc("TRN2", target_bir_lowering=False, debug=False,
                   num_devices=N_CORES)

    # ---- DRAM I/O (per core; natural layouts) ----
    # x rows are n-major: row = n*T + t
    d_x = nc.dram_tensor("x", (NL * t_steps, D), BF, kind="ExternalInput").ap()
    d_a = nc.dram_tensor("A", (NL, H * P16N), BF, kind="ExternalInput").ap()
    # this core's K-chunk of [Wx | Wh | Wattn] (natural gate column order)
    d_wsh = nc.dram_tensor("Wsh", (128, 3, 4096), BF, kind="ExternalInput").ap()
    d_bias = nc.dram_tensor("bias", (1, 4096), BF, kind="ExternalInput").ap()
    d_mdiag = nc.dram_tensor("mask_diag", (NL, NL * P16N), F32, kind="ExternalInput").ap()
    d_p16 = nc.dram_tensor("P16", (16, 128), BF, kind="ExternalInput").ap()
    d_mbd = nc.dram_tensor("mask_bd", (128, 4, NL), F32, kind="ExternalInput").ap()
    d_ident = nc.dram_tensor("ident", (128, 128), BF, kind="ExternalInput").ap()
    d_red4 = nc.dram_tensor("red4", (128, 32), BF, kind="ExternalInput").ap()
    d_hs = nc.dram_tensor("hs", (NL, t_steps, H), BF, kind="ExternalOutput").ap()

    MT = (NL * t_steps) // 128   # xproj M-tiles (n-major rows)

    with tile.TileContext(nc) as tc:
        with tc.tile_pool(name="resident", bufs=1) as resident, \
             tc.tile_pool(name="dram", bufs=1, space="DRAM") as dram:
            xp_d = dram.tile([NL * t_steps, NB, 512], BF)

            # ---------- weight AllGather (K-sharded upload) ----------
            # one combined gather: per-op overhead is high and effective
            # bandwidth improves with transfer size
            wbounce = dram.tile([128, 3 * 4096], BF, name="wbounce")
            wgfull = dram.tile([H, 3 * 4096], BF, name="wgfull")
            nc.gpsimd.dma_start(out=wbounce, in_=d_wsh)
            nc.gpsimd.collective_compute(
                "AllGather",
                ALU.bypass,
                replica_groups=[list(range(N_CORES))],
                ins=[wbounce.opt()],
                outs=[wgfull.opt()],
            )
            WROW = 3 * 4096   # gathered row stride; weight w at col w*4096

            # ---------- resident tiles ----------
            ident = resident.tile([128, 128], BF)
            nc.sync.dma_start(out=ident, in_=d_ident)
            mdiag = resident.tile([NL, NL * P16N], F32)
            nc.sync.dma_start(out=mdiag, in_=d_mdiag)
            p16 = resident.tile([16, 128], BF)
            nc.sync.dma_start(out=p16, in_=d_p16)
            mbd = resident.tile([128, 4, NL], F32)
            nc.sync.dma_start(out=mbd, in_=d_mbd)
            w_pad = resident.tile([NL, 32], BF)
            nc.vector.memset(w_pad, 0.0)
            red4 = resident.tile([128, 32], BF)
            nc.sync.dma_start(out=red4, in_=d_red4)

            a_r = resident.tile([128, HC, NL * P16N], BF)
            g_r = resident.tile([128, 4, 4096], BF)
            bias_aug = resident.tile([128, 4096], BF)
            nc.vector.memset(bias_aug, 0.0)
            nc.sync.dma_start(out=bias_aug[0:1, :], in_=d_bias)

            with tc.tile_pool(name="state", bufs=2) as state:

                # ---------- phase A: A-derived tensors ----------
                with tc.tile_pool(name="apre", bufs=1) as apre, \
                     tc.tile_pool(name="tps", bufs=4, space="PSUM") as tps:
                    a_sb = apre.tile([NL, H * P16N], BF, tag="a_sb")
                    nc.sync.dma_start(out=a_sb, in_=d_a)
                    arp = a_sb.ap[0][0]   # partition stride of a_sb
                    for hc in range(HC):
                        for p in range(P16N):
                            ps = tps.tile([128, NL], BF, tag="tp",
                                          name=f"tpa_{hc}_{p}")
                            nc.tensor.transpose(
                                ps,
                                _ap(a_sb, hc * 2048 + p, [[arp, NL], [16, 128]]),
                                ident[0:NL, 0:NL])
                            # a_r[:, hc, n*16+p] = A[n, hc*128+:, p]
                            nc.scalar.copy(
                                _ap(a_r, hc * 512 + p,
                                    [[a_r.ap[0][0], 128], [16, NL]]),
                                ps)

                    # c0 = mean_p A  (n-major), state f32
                    c_cur = state.tile([NL, H], F32, tag="c")
                    csum = apre.tile([NL, H], F32, tag="csum")
                    nc.vector.reduce_sum(
                        csum, _ap(a_sb, 0, [[arp, NL], [16, H], [1, 16]]),
                        axis=AX.X)
                    nc.vector.tensor_scalar_mul(c_cur, csum, 1.0 / 16.0)

                    # hT0 from a_r: mean over p, per K-chunk
                    hT_cur = state.tile([128, HC, NL], BF, tag="hT")
                    for hc in range(HC):
                        rsum = apre.tile([128, NL], F32, tag="rsum",
                                         name=f"rsum_{hc}")
                        nc.vector.reduce_sum(
                            rsum,
                            _ap(a_r, hc * 512,
                                [[a_r.ap[0][0], 128], [16, NL], [1, 16]]),
                            axis=AX.X)
                        nc.scalar.activation(hT_cur[:, hc, :], rsum, AF.Copy,
                                             scale=1.0 / 16.0)

                # ---------- phases B-D share xt_full ----------
                xtp_cm = tc.tile_pool(name="xtp", bufs=1)
                xtp = xtp_cm.__enter__()
                xt_full = xtp.tile([128, 9, NL * t_steps], BF)

                # ---------- phase B: x^T tiles (PE transposes) ----------
                with tc.tile_pool(name="xload", bufs=3) as xload, \
                     tc.tile_pool(name="tpsb", bufs=4, space="PSUM") as tps:
                    for mt in range(MT):
                        # tile rows are t-major: row = t*NL + n, t = 4*mt + tl
                        x_sb = xload.tile([128, D], BF, tag="x_sb",
                                          name=f"xsb_{mt}")
                        for tl in range(4):
                            nc.sync.dma_start(
                                out=x_sb[32 * tl:32 * (tl + 1), :],
                                in_=_ap(d_x, (4 * mt + tl) * D,
                                        [[t_steps * D, NL], [1, D]]))
                        for kc in range(8):
                            ps = tps.tile([128, 128], BF, tag="tp",
                                          name=f"tpx_{mt}_{kc}")
                            nc.tensor.transpose(
                                ps, x_sb[:, kc * 128:(kc + 1) * 128], ident)
                            nc.scalar.copy(
                                xt_full[:, kc, mt * 128:(mt + 1) * 128], ps)
                nc.vector.memset(xt_full[:, 8, :], 0.0)
                nc.vector.memset(xt_full[0:1, 8, :], 1.0)

                # ---------- phase C: G = A^T @ Wattn ----------
                with tc.tile_pool(name="gpre", bufs=3) as gpre, \
                     tc.tile_pool(name="gps", bufs=4, space="PSUM") as gps:
                    for nb in range(NB):
                        psum_g = [gps.tile([128, 512], F32, tag="gq",
                                           name=f"psum_g{nb}_{q}")
                                  for q in range(4)]
                        for kc in range(HC):
                            wa_t = gpre.tile([128, 512], BF, tag="wa")
                            nc.sync.dma_start(
                                out=wa_t,
                                in_=_ap(wgfull,
                                        kc * 128 * WROW + 2 * 4096 + nb * 512,
                                        [[WROW, 128], [1, 512]]))
                            for q in range(4):
                                nc.tensor.matmul(
                                    psum_g[q],
                                    lhsT=a_r[:, kc, 128 * q:128 * (q + 1)],
                                    rhs=wa_t,
                                    start=(kc == 0), stop=(kc == HC - 1))
                        for q in range(4):
                            nc.scalar.copy(g_r[:, q, nb * 512:(nb + 1) * 512],
                                           psum_g[q])

                # ---------- phase D: xproj = x @ Wx + b ----------
                with tc.tile_pool(name="xwc", bufs=2) as xwc, \
                     tc.tile_pool(name="xout", bufs=4) as xout, \
                     tc.tile_pool(name="xps", bufs=4, space="PSUM") as xps:
                    for nb in range(NB):
                        wxc = xwc.tile([128, 8, 512], BF, tag="wxc")
                        nc.sync.dma_start(
                            out=wxc,
                            in_=_ap(wgfull, nb * 512,
                                    [[WROW, 128], [128 * WROW, 8], [1, 512]]))
                        for mt in range(MT):
                            psum_xp = xps.tile([128, 512], F32, tag="xp")
                            for kc in range(9):
                                rhs = (wxc[:, kc, :] if kc < 8
                                       else bias_aug[:, nb * 512:(nb + 1) * 512])
                                nc.tensor.matmul(
                                    psum_xp,
                                    lhsT=xt_full[:, kc, mt * 128:(mt + 1) * 128],
                                    rhs=rhs,
                                    start=(kc == 0), stop=(kc == 8))
                            xo = xout.tile([128, 512], BF, tag="xo")
                            nc.scalar.copy(xo, psum_xp)
                            nc.sync.dma_start(
                                out=xp_d[mt * 128:(mt + 1) * 128, nb, :], in_=xo)

                xtp_cm.__exit__(None, None, None)

                # ---------- phase E: recurrence ----------
                wh_s = resident.tile([128, HC, 4096], BF)
                nc.sync.dma_start(
                    out=wh_s,
                    in_=_ap(wgfull, 1 * 4096,
                            [[WROW, 128], [128 * WROW, 8], [1, 4096]]))

                with tc.tile_pool(name="xpin", bufs=10) as xpin, \
                     tc.tile_pool(name="work", bufs=2) as work, \
                     tc.tile_pool(name="acts", bufs=2) as acts, \
                     tc.tile_pool(name="hout", bufs=3) as hout, \
                     tc.tile_pool(name="ps_we", bufs=1, space="PSUM") as ps_we, \
                     tc.tile_pool(name="ps_w", bufs=4, space="PSUM") as ps_w, \
                     tc.tile_pool(name="ps_pre", bufs=3, space="PSUM") as ps_pre:

                    # Column-group j of a col-tiled matmul takes K-chunks
                    # {j, 4+j}: round-0 work needs only the FIRST half of
                    # h^T, so all round-0 PE work is emitted before any
                    # round-1 work (the PE queue is in-order) and the PE
                    # stays busy through the previous step's cell-update
                    # tail -- the HAM throttle never re-engages.  h^T
                    # transposes run on the PE at the end of the step for
                    # the same reason.  Odd chunks are ordered so the
                    # half-1 cell chain's inputs (i=1, g=7, f=3) are ready
                    # earliest.
                    EVEN, ODD = [0, 2, 4, 6], [3, 1, 7, 5]

                    def _hpart_round(pw, r, nbs):
                        for j in range(4):
                            kc = 4 * r + j
                            nc.tensor.matmul(
                                pw[32 * j:32 * (j + 1), :],
                                lhsT=hT_cur[:, kc, :],
                                rhs=wh_s[:, kc, nbs],
                                start=(r == 0), stop=False,
                                tile_position=(0, 32 * j))

                    for t in range(t_steps):
                        xp_t = []
                        for nb in range(NB):
                            xpc = xpin.tile([NL, 512], BF, tag="xp_t",
                                            name=f"xp_{t}_{nb}")
                            # xp_d rows are t-major: row = t*NL + n
                            nc.sync.dma_start(
                                out=xpc, in_=xp_d[t * NL:(t + 1) * NL, nb, :])
                            xp_t.append(xpc)

                        # --- PE round 0 (needs only half-0 of h^T) ---
                        psum_scw = ps_w.tile([128, NL * P16N], F32, tag="w",
                                             name=f"scw_{t}")
                        for j in range(4):
                            nc.tensor.matmul(
                                psum_scw[32 * j:32 * (j + 1), :],
                                lhsT=hT_cur[:, j, :], rhs=a_r[:, j, :],
                                start=True, stop=False,
                                tile_position=(0, 32 * j))
                        psum_ws = {}
                        for nb in EVEN:
                            pw = ps_w.tile([128, 512], F32, tag="w",
                                           name=f"gw_{t}_{nb}")
                            psum_ws[nb] = pw
                            _hpart_round(pw, 0, slice(nb * 512, (nb + 1) * 512))

                        # --- PE round 1 (needs half-1 of h^T) ---
                        for j in range(4):
                            nc.tensor.matmul(
                                psum_scw[32 * j:32 * (j + 1), :],
                                lhsT=hT_cur[:, 4 + j, :], rhs=a_r[:, 4 + j, :],
                                start=False, stop=True,
                                tile_position=(0, 32 * j))
                        sb_scw = work.tile([128, NL * P16N], BF, tag="scw",
                                           name=f"sbscw_{t}")
                        nc.vector.tensor_copy(sb_scw, psum_scw)
                        psum_sc = ps_pre.tile([NL, NL * P16N], F32, tag="psc",
                                              bufs=1, name=f"psc_{t}")
                        nc.tensor.matmul(psum_sc, lhsT=red4, rhs=sb_scw,
                                         start=True, stop=True)
                        for nb in EVEN:
                            _hpart_round(psum_ws[nb], 1,
                                         slice(nb * 512, (nb + 1) * 512))

                        # --- softmax over the 16 positions (DVE/ACT;
                        #     overlaps the round-1 matmuls above) ---
                        masked = work.tile([NL, NL * P16N], F32, tag="masked")
                        nc.vector.tensor_mul(masked, psum_sc, mdiag)
                        masked_pm = _ap(masked, 0,
                                        [[masked.ap[0][0], NL], [1, P16N], [P16N, NL]])
                        scores32 = work.tile([NL, P16N], F32, tag="scores")
                        nc.vector.reduce_sum(scores32, masked_pm, axis=AX.X)
                        nmx = work.tile([NL, 1], F32, tag="nmx")
                        nc.vector.reduce_max(nmx, scores32, axis=AX.X, negate=True)
                        nmx_s = work.tile([NL, 1], F32, tag="nmx_s")
                        nc.vector.tensor_scalar_mul(nmx_s, nmx, SCALE)
                        # exp(x) = sig(x)/(1-sig(x)) for x<=0; stays in the
                        # sigmoid ACT table set (no per-step table reload)
                        sg = work.tile([NL, P16N], F32, tag="sg")
                        nc.scalar.activation(sg, scores32, AF.Sigmoid,
                                             bias=nmx_s, scale=SCALE)
                        om = work.tile([NL, P16N], F32, tag="om")
                        nc.vector.tensor_scalar(om, sg, -1.0, 1.0,
                                                ALU.mult, ALU.add)
                        omr = work.tile([NL, P16N], F32, tag="omr")
                        nc.vector.reciprocal(omr, om)
                        e_t = work.tile([NL, P16N], F32, tag="e")
                        nc.vector.tensor_mul(e_t, sg, omr)
                        ssum = work.tile([NL, 1], F32, tag="ssum")
                        nc.vector.reduce_sum(ssum, e_t, axis=AX.X)
                        rr = work.tile([NL, 1], F32, tag="rr")
                        nc.vector.reciprocal(rr, ssum)
                        nc.vector.tensor_scalar_mul(w_pad[:, 0:P16N], e_t, rr)
                        wT_t = work.tile([NL, 32], BF, tag="wT")
                        nc.vector.transpose(out=wT_t, in_=w_pad)
                        psum_we = ps_we.tile([128, NL], F32, tag="we")
                        nc.tensor.matmul(psum_we, lhsT=p16, rhs=wT_t[0:16, :],
                                         start=True, stop=True)
                        w_bdb = work.tile([128, 4, NL], BF, tag="wbd")
                        wexp_b = _ap(psum_we, 0,
                                     [[psum_we.ap[0][0], 128], [0, 4], [1, NL]])
                        nc.vector.tensor_mul(w_bdb, wexp_b, mbd)

                        # --- finish gate chunks: attention round, fold
                        #     column-group partials + xproj (red4 matmul, xp
                        #     added to group 0 during the PSUM->SBUF move),
                        #     then the gate nonlinearity (chunks 6,7 = tanh)
                        gacts = [None] * NB
                        c_nxt = state.tile([NL, H], F32, tag="c")
                        h_bf = hout.tile([NL, H], BF, tag="hbf")
                        hT_nxt = state.tile([128, HC, NL], BF, tag="hT")

                        def _cell_half(j):
                            si, sf, so, tg = (gacts[j], gacts[2 + j],
                                              gacts[4 + j], gacts[6 + j])
                            jsl = slice(j * 512, (j + 1) * 512)
                            t1 = acts.tile([NL, 512], F32, tag="ct", bufs=14,
                                           name=f"t1_{t}_{j}")
                            nc.vector.tensor_mul(t1, sf, c_cur[:, jsl])
                            t2 = acts.tile([NL, 512], F32, tag="ct", bufs=14,
                                           name=f"t2_{t}_{j}")
                            nc.vector.tensor_mul(t2, si, tg)
                            nc.vector.tensor_add(c_nxt[:, jsl], t1, t2)
                            tcn = acts.tile([NL, 512], F32, tag="ct", bufs=14,
                                            name=f"tcn_{t}_{j}")
                            nc.scalar.activation(tcn, c_nxt[:, jsl], AF.Tanh)
                            nc.vector.tensor_mul(h_bf[:, jsl], so, tcn)

                        def _transpose_half(j):
                            # h^T for this half on the PE (keeps it warm
                            # through the cell-update tail)
                            for hc in range(4 * j, 4 * j + 4):
                                tp = ps_w.tile([128, NL], BF, tag="w",
                                               name=f"tp_{t}_{hc}")
                                nc.tensor.transpose(
                                    tp, h_bf[:, hc * 128:(hc + 1) * 128],
                                    ident[0:NL, 0:NL])
                                nc.scalar.copy(hT_nxt[:, hc, :], tp)

                        def _finish_nb(nb):
                            nbs = slice(nb * 512, (nb + 1) * 512)
                            pw = psum_ws[nb]
                            for j in range(4):
                                nc.tensor.matmul(
                                    pw[32 * j:32 * (j + 1), :],
                                    lhsT=w_bdb[:, j, :],
                                    rhs=g_r[:, j, nbs],
                                    start=False, stop=True,
                                    tile_position=(0, 32 * j))
                            sb_w = work.tile([128, 512], BF, tag="gw", bufs=4,
                                             name=f"sbw_{t}_{nb}")
                            # xp folds into group 0's partial during the
                            # PSUM->SBUF move; partition-range rule: APs
                            # based at partition 32/64 span <=32/64 rows.
                            nc.vector.tensor_add(sb_w[0:NL, :], pw[0:NL, :],
                                                 xp_t[nb])
                            if nb % 2 == 0:
                                nc.scalar.copy(sb_w[NL:2 * NL, :],
                                               pw[NL:2 * NL, :])
                                nc.scalar.copy(sb_w[2 * NL:128, :],
                                               pw[2 * NL:128, :])
                            else:
                                nc.vector.tensor_copy(sb_w[NL:2 * NL, :],
                                                      pw[NL:2 * NL, :])
                                nc.scalar.copy(sb_w[2 * NL:128, :],
                                               pw[2 * NL:128, :])
                            psum_pre = ps_pre.tile([NL, 512], F32, tag="pre",
                                                   bufs=2, name=f"pre_{t}_{nb}")
                            nc.tensor.matmul(psum_pre, lhsT=red4, rhs=sb_w,
                                             start=True, stop=True)
                            ga = acts.tile([NL, 512], F32, tag="ct", bufs=14,
                                           name=f"ga_{t}_{nb}")
                            nc.scalar.activation(
                                ga, psum_pre,
                                AF.Tanh if nb >= 6 else AF.Sigmoid)
                            gacts[nb] = ga

                        for nb in EVEN:
                            _finish_nb(nb)
                        _cell_half(0)
                        for nb in ODD:
                            pw = ps_w.tile([128, 512], F32, tag="w",
                                           name=f"gw_{t}_{nb}")
                            psum_ws[nb] = pw
                            nbs = slice(nb * 512, (nb + 1) * 512)
                            _hpart_round(pw, 0, nbs)
                            _hpart_round(pw, 1, nbs)
                            _finish_nb(nb)
                        _cell_half(1)
                        _transpose_half(0)
                        _transpose_half(1)
                        nc.sync.dma_start(out=d_hs[:, t, :], in_=h_bf)

                        hT_cur = hT_nxt
                        c_cur = c_nxt

    nc.compile()
    return nc


_PROGRAM_CACHE = {}


def _get_program(t_steps=T):
    if t_steps not in _PROGRAM_CACHE:
        _PROGRAM_CACHE[t_steps] = _build_program(t_steps)
    return _PROGRAM_CACHE[t_steps]


# ---------------------------------------------------------------------------
# Host-side constants (input-independent), cached at module level.
# ---------------------------------------------------------------------------
def _consts():
    mask_diag = np.zeros((NL, NL * P16N), dtype=np.float32)
    for i in range(NL):
        mask_diag[i, i * P16N:(i + 1) * P16N] = 1.0
    p16_m = np.zeros((16, 128), dtype=BF16)
    for j in range(128):
        p16_m[j % 16, j] = 1.0
    mask_bd = np.zeros((128, 4, NL), dtype=np.float32)
    for part in range(128):
        n_lo = part // 16
        for q in range(4):
            mask_bd[part, q, 8 * q + n_lo] = 1.0
    ident = np.eye(128, dtype=BF16)
    red4 = np.tile(np.eye(32, dtype=BF16), (4, 1))
    return {
        "mask_diag": np.tile(mask_diag, (N_CORES, 1)),
        "P16": np.tile(p16_m, (N_CORES, 1)),
        "mask_bd": np.tile(mask_bd, (N_CORES, 1, 1)),
        "ident": np.tile(ident, (N_CORES, 1)),
        "red4": np.tile(red4, (N_CORES, 1)),
    }


_CONSTS = None


def _get_consts():
    global _CONSTS
    if _CONSTS is None:
        _CONSTS = _consts()
    return _CONSTS


# ---------------------------------------------------------------------------
# Cached PJRT runner (jit built once per program).
# ---------------------------------------------------------------------------
_RUNNER_CACHE = {}


def _get_runner(nc):
    key = id(nc)
    if key in _RUNNER_CACHE:
        return _RUNNER_CACHE[key]

    import jax
    import jax.numpy as jnp
    from jax.sharding import Mesh, PartitionSpec, NamedSharding
    from jax.experimental.shard_map import shard_map
    from concourse.bass2jax import (
        install_neuronx_cc_hook, _bass_exec_p, partition_id_tensor)

    install_neuronx_cc_hook()

    partition_name = (nc.partition_id_tensor.name
                      if nc.partition_id_tensor else None)
    in_names, out_names, out_avals = [], [], []
    for alloc in nc.m.functions[0].allocations:
        if not isinstance(alloc, mybir.MemoryLocationSet):
            continue
        name = alloc.memorylocations[0].name
        if alloc.kind == "ExternalInput":
            if name != partition_name:
                in_names.append(name)
        elif alloc.kind == "ExternalOutput":
            out_names.append(name)
            out_avals.append(jax.core.ShapedArray(
                tuple(alloc.tensor_shape), mybir.dt.np(alloc.dtype)))
    n_params = len(in_names)
    all_in_names = list(in_names) + list(out_names)
    if partition_name is not None:
        all_in_names.append(partition_name)

    def _body(*args):
        operands = list(args)
        if partition_name is not None:
            operands.append(partition_id_tensor())
        outs = _bass_exec_p.bind(
            *operands,
            out_avals=tuple(out_avals),
            in_names=tuple(all_in_names),
            out_names=tuple(out_names),
            lowering_input_output_aliases=(),
            sim_require_finite=True,
            sim_require_nnan=True,
            nc=nc,
        )
        return tuple(outs)

    devices = jax.devices()[:N_CORES]
    mesh = Mesh(np.asarray(devices), ("core",))
    n_outs = len(out_names)
    in_specs = (PartitionSpec("core"),) * (n_params + n_outs)
    out_specs = (PartitionSpec("core"),) * n_outs
    donate = tuple(range(n_params, n_params + n_outs))
    sharded = jax.jit(
        shard_map(_body, mesh=mesh, in_specs=in_specs,
                  out_specs=out_specs, check_rep=False),
        donate_argnums=donate, keep_unused=True)
    zero_sh = tuple(NamedSharding(mesh, PartitionSpec("core"))
                    for _ in out_avals)
    make_zeros = jax.jit(
        lambda: tuple(jnp.zeros((N_CORES * av.shape[0], *av.shape[1:]),
                                av.dtype) for av in out_avals),
        out_shardings=zero_sh)

    def run(global_in: dict):
        args = [global_in[name] for name in in_names]
        out_arrs = sharded(*args, *make_zeros())
        return {name: out_arrs[i] for i, name in enumerate(out_names)}

    _RUNNER_CACHE[key] = run
    return run


_PREP_CACHE = {}


def _get_prep():
    if "prep" in _PREP_CACHE:
        return _PREP_CACHE["prep"]
    import jax
    import jax.numpy as jnp
    cpu = jax.devices("cpu")[0]
    bf = jnp.bfloat16

    def _prep(x, A, Wx, Wh, Wattn, b):
        xg = x.reshape(N * T, D).astype(bf)
        ag = A.reshape(N, H * P16N).astype(bf)
        wg = jnp.stack([Wx, Wh, Wattn], axis=1).astype(bf)
        bg = jnp.broadcast_to(b.astype(bf)[None, :], (N_CORES, 4 * H))
        return xg, ag, wg, bg

    def _post(hs):
        return hs.astype(jnp.float32)

    prep = (jax.jit(_prep, device=cpu), jax.jit(_post, device=cpu))
    _PREP_CACHE["prep"] = prep
    return prep


def kernel(**inputs):
    x = np.ascontiguousarray(np.asarray(inputs["x"], dtype=np.float32))
    A = np.ascontiguousarray(np.asarray(inputs["A"], dtype=np.float32))
    Wx = np.asarray(inputs["Wx"], dtype=np.float32)
    Wh = np.asarray(inputs["Wh"], dtype=np.float32)
    Wattn = np.asarray(inputs["Wattn"], dtype=np.float32)
    b = np.asarray(inputs["b"], dtype=np.float32)

    nc = _get_program(T)
    run = _get_runner(nc)
    prep, post = _get_prep()
    xg, ag, wgl, bg = (np.asarray(v) for v in prep(x, A, Wx, Wh, Wattn, b))
    consts = _get_consts()
    global_in = {
        "x": xg, "A": ag, "Wsh": wgl, "bias": bg,
        "mask_diag": consts["mask_diag"], "P16": consts["P16"],
        "mask_bd": consts["mask_bd"], "ident": consts["ident"],
        "red4": consts["red4"],
    }
    outs = run(global_in)
    hs = np.asarray(post(outs["hs"]))          # (256, 64, 1024) f32
    return hs.reshape(N, T, H)


# revision 23
# speedup vs baseline: 1.2448x; 1.0157x over previous
"""AttentionLSTM Trainium2 kernel (8-core data-parallel, Bass/Tile).

Reference computation (per step t, batch N=256, H=D=1024):
    scores = einsum('nh,nhp->np', h, A_flat) / sqrt(H)
    w      = softmax(scores)                      # over 16 spatial positions
    attn   = einsum('nhp,np->nh', A_flat, w)
    a      = x_t @ Wx + h @ Wh + attn @ Wattn + b
    i,f,o,g = split(a, 4); c = sig(f)*c + sig(i)*tanh(g); h = sig(o)*tanh(c)

Mapping:
  * Data-parallel: batch 256 -> 8 cores x 32.
  * Inputs ship in natural layout (bf16); all layout transformation
    happens on device (PE transposes).  Weights ship K-sharded (each
    core uploads a distinct 128-row chunk of Wx/Wh/Wattn) and are
    reassembled on device with a NeuronLink AllGather, cutting host->
    device traffic ~8x for the replicated weights.
  * xproj = x @ Wx + b hoisted out of the scan (bias folded in as a
    ones-row K-chunk), staged through DRAM scratch in bf16.
  * attn @ Wattn re-associated: G[(n,p),:] = A[n,:,p] @ Wattn once,
    per step the attention contribution is w_blockdiag.T @ G.
  * scores via a cross-product matmul + masked diagonal extract.
  * softmax exp computed as sig(x)/(1-sig(x)) (x <= 0 post max-sub) so
    the recurrence stays in one ACT table set (no per-step table loads).
  * All matmul operands bf16; accumulation/state f32; output bf16.
"""
import sys

sys.path.insert(0, "/opt/trn_rl_repo")

import numpy as np
import ml_dtypes

import concourse.bacc as bacc
import concourse.bass as bass
import concourse.tile as tile
from concourse import mybir

BF16 = ml_dtypes.bfloat16
F32 = mybir.dt.float32
BF = mybir.dt.bfloat16
AF = mybir.ActivationFunctionType
AX = mybir.AxisListType
ALU = mybir.AluOpType

N_CORES = 8
N, T, D, H = 256, 64, 1024, 1024
NL = N // N_CORES            # 32 batch per core
HC = H // 128                # 8 K-chunks of the hidden dim
NB = 8                       # 512-wide gate column chunks
P16N = 16                    # attention positions
SCALE = 1.0 / float(np.sqrt(H))


def _ap(t, off, dims):
    """AP over tile/AP `t` at element offset `off` with dims [[stride, n], ...]."""
    return bass.AP(tensor=t.tensor, offset=t.offset + off, ap=dims)


def _build_program(t_steps=T):
    nc = bacc.Bacc("TRN2", target_bir_lowering=False, debug=False,
                   num_devices=N_CORES)

    # ---- DRAM I/O (per core; natural layouts) ----
    # x rows are n-major: row = n*T + t
    d_x = nc.dram_tensor("x", (NL * t_steps, D), BF, kind="ExternalInput").ap()
    d_a = nc.dram_tensor("A", (NL, H * P16N), BF, kind="ExternalInput").ap()
    # this core's K-chunk of [Wx | Wh | Wattn] (natural gate column order)
    d_wsh = nc.dram_tensor("Wsh", (128, 3, 4096), BF, kind="ExternalInput").ap()
    d_bias = nc.dram_tensor("bias", (1, 4096), BF, kind="ExternalInput").ap()
    d_mdiag = nc.dram_tensor("mask_diag", (NL, NL * P16N), F32, kind="ExternalInput").ap()
    d_p16 = nc.dram_tensor("P16", (16, 128), BF, kind="ExternalInput").ap()
    d_mbd = nc.dram_tensor("mask_bd", (128, 4, NL), F32, kind="ExternalInput").ap()
    d_ident = nc.dram_tensor("ident", (128, 128), BF, kind="ExternalInput").ap()
    d_red4 = nc.dram_tensor("red4", (128, 32), BF, kind="ExternalInput").ap()
    d_hs = nc.dram_tensor("hs", (NL, t_steps, H), BF, kind="ExternalOutput").ap()

    MT = (NL * t_steps) // 128   # xproj M-tiles (n-major rows)

    with tile.TileContext(nc) as tc:
        with tc.tile_pool(name="resident", bufs=1) as resident, \
             tc.tile_pool(name="dram", bufs=1, space="DRAM") as dram:
            xp_d = dram.tile([NL * t_steps, NB, 512], BF)

            # ---------- weight AllGather (K-sharded upload) ----------
            # one combined gather: per-op overhead is high and effective
            # bandwidth improves with transfer size
            wbounce = dram.tile([128, 3 * 4096], BF, name="wbounce")
            wgfull = dram.tile([H, 3 * 4096], BF, name="wgfull")
            nc.gpsimd.dma_start(out=wbounce, in_=d_wsh)
            nc.gpsimd.collective_compute(
                "AllGather",
                ALU.bypass,
                replica_groups=[list(range(N_CORES))],
                ins=[wbounce.opt()],
                outs=[wgfull.opt()],
            )
            WROW = 3 * 4096   # gathered row stride; weight w at col w*4096

            # ---------- resident tiles ----------
            ident = resident.tile([128, 128], BF)
            nc.sync.dma_start(out=ident, in_=d_ident)
            mdiag = resident.tile([NL, NL * P16N], F32)
            nc.sync.dma_start(out=mdiag, in_=d_mdiag)
            p16 = resident.tile([16, 128], BF)
            nc.sync.dma_start(out=p16, in_=d_p16)
            mbd = resident.tile([128, 4, NL], F32)
            nc.sync.dma_start(out=mbd, in_=d_mbd)
            w_pad = resident.tile([NL, 32], BF)
            nc.vector.memset(w_pad, 0.0)
            red4 = resident.tile([128, 32], BF)
            nc.sync.dma_start(out=red4, in_=d_red4)

            a_r = resident.tile([128, HC, NL * P16N], BF)
            g_r = resident.tile([128, 4, 4096], BF)
            bias_aug = resident.tile([128, 4096], BF)
            nc.vector.memset(bias_aug, 0.0)
            nc.sync.dma_start(out=bias_aug[0:1, :], in_=d_bias)

            with tc.tile_pool(name="state", bufs=2) as state:

                # ---------- phase A: A-derived tensors ----------
                with tc.tile_pool(name="apre", bufs=1) as apre, \
                     tc.tile_pool(name="tps", bufs=4, space="PSUM") as tps:
                    a_sb = apre.tile([NL, H * P16N], BF, tag="a_sb")
                    nc.sync.dma_start(out=a_sb, in_=d_a)
                    arp = a_sb.ap[0][0]   # partition stride of a_sb
                    for hc in range(HC):
                        for p in range(P16N):
                            ps = tps.tile([128, NL], BF, tag="tp",
                                          name=f"tpa_{hc}_{p}")
                            nc.tensor.transpose(
                                ps,
                                _ap(a_sb, hc * 2048 + p, [[arp, NL], [16, 128]]),
                                ident[0:NL, 0:NL])
                            # a_r[:, hc, n*16+p] = A[n, hc*128+:, p]
                            nc.scalar.copy(
                                _ap(a_r, hc * 512 + p,
                                    [[a_r.ap[0][0], 128], [16, NL]]),
                                ps)

                    # c0 = mean_p A  (n-major), state f32
                    c_cur = state.tile([NL, H], F32, tag="c")
                    csum = apre.tile([NL, H], F32, tag="csum")
                    nc.vector.reduce_sum(
                        csum, _ap(a_sb, 0, [[arp, NL], [16, H], [1, 16]]),
                        axis=AX.X)
                    nc.vector.tensor_scalar_mul(c_cur, csum, 1.0 / 16.0)

                    # hT0 from a_r: mean over p, per K-chunk
                    hT_cur = state.tile([128, HC, NL], BF, tag="hT")
                    for hc in range(HC):
                        rsum = apre.tile([128, NL], F32, tag="rsum",
                                         name=f"rsum_{hc}")
                        nc.vector.reduce_sum(
                            rsum,
                            _ap(a_r, hc * 512,
                                [[a_r.ap[0][0], 128], [16, NL], [1, 16]]),
                            axis=AX.X)
                        nc.scalar.activation(hT_cur[:, hc, :], rsum, AF.Copy,
                                             scale=1.0 / 16.0)

                # ---------- phases B-D share xt_full ----------
                xtp_cm = tc.tile_pool(name="xtp", bufs=1)
                xtp = xtp_cm.__enter__()
                xt_full = xtp.tile([128, 9, NL * t_steps], BF)

                # ---------- phase B: x^T tiles (PE transposes) ----------
                with tc.tile_pool(name="xload", bufs=3) as xload, \
                     tc.tile_pool(name="tpsb", bufs=4, space="PSUM") as tps:
                    for mt in range(MT):
                        # tile rows are t-major: row = t*NL + n, t = 4*mt + tl
                        x_sb = xload.tile([128, D], BF, tag="x_sb",
                                          name=f"xsb_{mt}")
                        for tl in range(4):
                            nc.sync.dma_start(
                                out=x_sb[32 * tl:32 * (tl + 1), :],
                                in_=_ap(d_x, (4 * mt + tl) * D,
                                        [[t_steps * D, NL], [1, D]]))
                        for kc in range(8):
                            ps = tps.tile([128, 128], BF, tag="tp",
                                          name=f"tpx_{mt}_{kc}")
                            nc.tensor.transpose(
                                ps, x_sb[:, kc * 128:(kc + 1) * 128], ident)
                            nc.scalar.copy(
                                xt_full[:, kc, mt * 128:(mt + 1) * 128], ps)
                nc.vector.memset(xt_full[:, 8, :], 0.0)
                nc.vector.memset(xt_full[0:1, 8, :], 1.0)

                # ---------- phase C: G = A^T @ Wattn ----------
                with tc.tile_pool(name="gpre", bufs=3) as gpre, \
                     tc.tile_pool(name="gps", bufs=4, space="PSUM") as gps:
                    for nb in range(NB):
                        psum_g = [gps.tile([128, 512], F32, tag="gq",
                                           name=f"psum_g{nb}_{q}")
                                  for q in range(4)]
                        for kc in range(HC):
                            wa_t = gpre.tile([128, 512], BF, tag="wa")
                            nc.sync.dma_start(
                                out=wa_t,
                                in_=_ap(wgfull,
                                        kc * 128 * WROW + 2 * 4096 + nb * 512,
                                        [[WROW, 128], [1, 512]]))
                            for q in range(4):
                                nc.tensor.matmul(
                                    psum_g[q],
                                    lhsT=a_r[:, kc, 128 * q:128 * (q + 1)],
                                    rhs=wa_t,
                                    start=(kc == 0), stop=(kc == HC - 1))
                        for q in range(4):
                            nc.scalar.copy(g_r[:, q, nb * 512:(nb + 1) * 512],
                                           psum_g[q])

                # ---------- phase D: xproj = x @ Wx + b ----------
                with tc.tile_pool(name="xwc", bufs=2) as xwc, \
                     tc.tile_pool(name="xout", bufs=4) as xout, \
                     tc.tile_pool(name="xps", bufs=4, space="PSUM") as xps:
                    for nb in range(NB):
                        wxc = xwc.tile([128, 8, 512], BF, tag="wxc")
                        nc.sync.dma_start(
                            out=wxc,
                            in_=_ap(wgfull, nb * 512,
                                    [[WROW, 128], [128 * WROW, 8], [1, 512]]))
                        for mt in range(MT):
                            psum_xp = xps.tile([128, 512], F32, tag="xp")
                            for kc in range(9):
                                rhs = (wxc[:, kc, :] if kc < 8
                                       else bias_aug[:, nb * 512:(nb + 1) * 512])
                                nc.tensor.matmul(
                                    psum_xp,
                                    lhsT=xt_full[:, kc, mt * 128:(mt + 1) * 128],
                                    rhs=rhs,
                                    start=(kc == 0), stop=(kc == 8))
                            xo = xout.tile([128, 512], BF, tag="xo")
                            nc.scalar.copy(xo, psum_xp)
                            nc.sync.dma_start(
                                out=xp_d[mt * 128:(mt + 1) * 128, nb, :], in_=xo)

                xtp_cm.__exit__(None, None, None)

                # ---------- phase E: recurrence ----------
                wh_s = resident.tile([128, HC, 4096], BF)
                nc.sync.dma_start(
                    out=wh_s,
                    in_=_ap(wgfull, 1 * 4096,
                            [[WROW, 128], [128 * WROW, 8], [1, 4096]]))

                with tc.tile_pool(name="xpin", bufs=10) as xpin, \
                     tc.tile_pool(name="work", bufs=2) as work, \
                     tc.tile_pool(name="acts", bufs=2) as acts, \
                     tc.tile_pool(name="hout", bufs=3) as hout, \
                     tc.tile_pool(name="ps_we", bufs=1, space="PSUM") as ps_we, \
                     tc.tile_pool(name="ps_w", bufs=4, space="PSUM") as ps_w, \
                     tc.tile_pool(name="ps_pre", bufs=3, space="PSUM") as ps_pre:

                    # Column-group j of a col-tiled matmul takes K-chunks
                    # {j, 4+j}: round-0 work needs only the FIRST half of
                    # h^T, so all round-0 PE work is emitted before any
                    # round-1 work (the PE queue is in-order) and the PE
                    # stays busy through the previous step's cell-update
                    # tail -- the HAM throttle never re-engages.  h^T
                    # transposes run on the PE at the end of the step for
                    # the same reason.  Odd chunks are ordered so the
                    # half-1 cell chain's inputs (i=1, g=7, f=3) are ready
                    # earliest.
                    EVEN, ODD = [0, 2, 4, 6], [3, 1, 7, 5]

                    def _hpart_round(pw, r, nbs):
                        for j in range(4):
                            kc = 4 * r + j
                            nc.tensor.matmul(
                                pw[32 * j:32 * (j + 1), :],
                                lhsT=hT_cur[:, kc, :],
                                rhs=wh_s[:, kc, nbs],
                                start=(r == 0), stop=False,
                                tile_position=(0, 32 * j))

                    for t in range(t_steps):
                        xp_t = []
                        for nb in range(NB):
                            xpc = xpin.tile([NL, 512], BF, tag="xp_t",
                                            name=f"xp_{t}_{nb}")
                            # xp_d rows are t-major: row = t*NL + n
                            nc.sync.dma_start(
                                out=xpc, in_=xp_d[t * NL:(t + 1) * NL, nb, :])
                            xp_t.append(xpc)

                        # --- PE round 0 (needs only half-0 of h^T) ---
                        psum_scw = ps_w.tile([128, NL * P16N], F32, tag="w",
                                             name=f"scw_{t}")
                        for j in range(4):
                            nc.tensor.matmul(
                                psum_scw[32 * j:32 * (j + 1), :],
                                lhsT=hT_cur[:, j, :], rhs=a_r[:, j, :],
                                start=True, stop=False,
                                tile_position=(0, 32 * j))
                        psum_ws = {}
                        for nb in EVEN:
                            pw = ps_w.tile([128, 512], F32, tag="w",
                                           name=f"gw_{t}_{nb}")
                            psum_ws[nb] = pw
                            _hpart_round(pw, 0, slice(nb * 512, (nb + 1) * 512))

                        # --- PE round 1 (needs half-1 of h^T) ---
                        for j in range(4):
                            nc.tensor.matmul(
                                psum_scw[32 * j:32 * (j + 1), :],
                                lhsT=hT_cur[:, 4 + j, :], rhs=a_r[:, 4 + j, :],
                                start=False, stop=True,
                                tile_position=(0, 32 * j))
                        sb_scw = work.tile([128, NL * P16N], BF, tag="scw",
                                           name=f"sbscw_{t}")
                        nc.vector.tensor_copy(sb_scw, psum_scw)
                        psum_sc = ps_pre.tile([NL, NL * P16N], F32, tag="psc",
                                              bufs=1, name=f"psc_{t}")
                        nc.tensor.matmul(psum_sc, lhsT=red4, rhs=sb_scw,
                                         start=True, stop=True)
                        for nb in EVEN:
                            _hpart_round(psum_ws[nb], 1,
                                         slice(nb * 512, (nb + 1) * 512))

                        # --- softmax over the 16 positions (DVE/ACT;
                        #     overlaps the round-1 matmuls above) ---
                        masked = work.tile([NL, NL * P16N], F32, tag="masked")
                        nc.vector.tensor_mul(masked, psum_sc, mdiag)
                        masked_pm = _ap(masked, 0,
                                        [[masked.ap[0][0], NL], [1, P16N], [P16N, NL]])
                        scores32 = work.tile([NL, P16N], F32, tag="scores")
                        nc.vector.reduce_sum(scores32, masked_pm, axis=AX.X)
                        # exp(x) = sig(x)/(1-sig(x)); no max-subtraction --
                        # h is tanh-bounded so |x| <~ 5 and 1-sig(x) keeps
                        # full f32 precision; stays in the sigmoid ACT
                        # table set (no per-step table reload).  Shortens
                        # the serial softmax chain the attn matmuls wait on.
                        sg = work.tile([NL, P16N], F32, tag="sg")
                        nc.scalar.activation(sg, scores32, AF.Sigmoid,
                                             scale=SCALE)
                        om = work.tile([NL, P16N], F32, tag="om")
                        nc.vector.tensor_scalar(om, sg, -1.0, 1.0,
                                                ALU.mult, ALU.add)
                        omr = work.tile([NL, P16N], F32, tag="omr")
                        nc.vector.reciprocal(omr, om)
                        e_t = work.tile([NL, P16N], F32, tag="e")
                        nc.vector.tensor_mul(e_t, sg, omr)
                        ssum = work.tile([NL, 1], F32, tag="ssum")
                        nc.vector.reduce_sum(ssum, e_t, axis=AX.X)
                        rr = work.tile([NL, 1], F32, tag="rr")
                        nc.vector.reciprocal(rr, ssum)
                        nc.vector.tensor_scalar_mul(w_pad[:, 0:P16N], e_t, rr)
                        wT_t = work.tile([NL, 32], BF, tag="wT")
                        nc.vector.transpose(out=wT_t, in_=w_pad)
                        psum_we = ps_we.tile([128, NL], F32, tag="we")
                        nc.tensor.matmul(psum_we, lhsT=p16, rhs=wT_t[0:16, :],
                                         start=True, stop=True)
                        w_bdb = work.tile([128, 4, NL], BF, tag="wbd")
                        wexp_b = _ap(psum_we, 0,
                                     [[psum_we.ap[0][0], 128], [0, 4], [1, NL]])
                        nc.vector.tensor_mul(w_bdb, wexp_b, mbd)

                        # --- finish gate chunks: attention round, fold
                        #     column-group partials + xproj (red4 matmul, xp
                        #     added to group 0 during the PSUM->SBUF move),
                        #     then the gate nonlinearity (chunks 6,7 = tanh)
                        gacts = [None] * NB
                        c_nxt = state.tile([NL, H], F32, tag="c")
                        h_bf = hout.tile([NL, H], BF, tag="hbf")
                        hT_nxt = state.tile([128, HC, NL], BF, tag="hT")

                        def _cell_half(j):
                            si, sf, so, tg = (gacts[j], gacts[2 + j],
                                              gacts[4 + j], gacts[6 + j])
                            jsl = slice(j * 512, (j + 1) * 512)
                            t1 = acts.tile([NL, 512], F32, tag="ct", bufs=14,
                                           name=f"t1_{t}_{j}")
                            nc.vector.tensor_mul(t1, sf, c_cur[:, jsl])
                            t2 = acts.tile([NL, 512], F32, tag="ct", bufs=14,
                                           name=f"t2_{t}_{j}")
                            nc.vector.tensor_mul(t2, si, tg)
                            nc.vector.tensor_add(c_nxt[:, jsl], t1, t2)
                            tcn = acts.tile([NL, 512], F32, tag="ct", bufs=14,
                                            name=f"tcn_{t}_{j}")
                            nc.scalar.activation(tcn, c_nxt[:, jsl], AF.Tanh)
                            nc.vector.tensor_mul(h_bf[:, jsl], so, tcn)

                        def _transpose_half(j):
                            # h^T for this half on the PE (keeps it warm
                            # through the cell-update tail)
                            for hc in range(4 * j, 4 * j + 4):
                                tp = ps_w.tile([128, NL], BF, tag="w",
                                               name=f"tp_{t}_{hc}")
                                nc.tensor.transpose(
                                    tp, h_bf[:, hc * 128:(hc + 1) * 128],
                                    ident[0:NL, 0:NL])
                                nc.scalar.copy(hT_nxt[:, hc, :], tp)

                        def _finish_nb(nb):
                            nbs = slice(nb * 512, (nb + 1) * 512)
                            pw = psum_ws[nb]
                            for j in range(4):
                                nc.tensor.matmul(
                                    pw[32 * j:32 * (j + 1), :],
                                    lhsT=w_bdb[:, j, :],
                                    rhs=g_r[:, j, nbs],
                                    start=False, stop=True,
                                    tile_position=(0, 32 * j))
                            sb_w = work.tile([128, 512], BF, tag="gw", bufs=4,
                                             name=f"sbw_{t}_{nb}")
                            # xp folds into group 0's partial during the
                            # PSUM->SBUF move; partition-range rule: APs
                            # based at partition 32/64 span <=32/64 rows.
                            nc.vector.tensor_add(sb_w[0:NL, :], pw[0:NL, :],
                                                 xp_t[nb])
                            if nb % 2 == 0:
                                nc.scalar.copy(sb_w[NL:2 * NL, :],
                                               pw[NL:2 * NL, :])
                                nc.scalar.copy(sb_w[2 * NL:128, :],
                                               pw[2 * NL:128, :])
                            else:
                                nc.vector.tensor_copy(sb_w[NL:2 * NL, :],
                                                      pw[NL:2 * NL, :])
                                nc.scalar.copy(sb_w[2 * NL:128, :],
                                               pw[2 * NL:128, :])
                            psum_pre = ps_pre.tile([NL, 512], F32, tag="pre",
                                                   bufs=2, name=f"pre_{t}_{nb}")
                            nc.tensor.matmul(psum_pre, lhsT=red4, rhs=sb_w,
                                             start=True, stop=True)
                            ga = acts.tile([NL, 512], F32, tag="ct", bufs=14,
                                           name=f"ga_{t}_{nb}")
                            nc.scalar.activation(
                                ga, psum_pre,
                                AF.Tanh if nb >= 6 else AF.Sigmoid)
                            gacts[nb] = ga

                        for nb in EVEN:
                            _finish_nb(nb)
                        _cell_half(0)
                        for nb in ODD:
                            pw = ps_w.tile([128, 512], F32, tag="w",
                                           name=f"gw_{t}_{nb}")
                            psum_ws[nb] = pw
                            nbs = slice(nb * 512, (nb + 1) * 512)
                            _hpart_round(pw, 0, nbs)
                            _hpart_round(pw, 1, nbs)
                            _finish_nb(nb)
                        _cell_half(1)
                        _transpose_half(0)
                        _transpose_half(1)
                        nc.sync.dma_start(out=d_hs[:, t, :], in_=h_bf)

                        hT_cur = hT_nxt
                        c_cur = c_nxt

    nc.compile()
    return nc


_PROGRAM_CACHE = {}


def _get_program(t_steps=T):
    if t_steps not in _PROGRAM_CACHE:
        _PROGRAM_CACHE[t_steps] = _build_program(t_steps)
    return _PROGRAM_CACHE[t_steps]


# ---------------------------------------------------------------------------
# Host-side constants (input-independent), cached at module level.
# ---------------------------------------------------------------------------
def _consts():
    mask_diag = np.zeros((NL, NL * P16N), dtype=np.float32)
    for i in range(NL):
        mask_diag[i, i * P16N:(i + 1) * P16N] = 1.0
    p16_m = np.zeros((16, 128), dtype=BF16)
    for j in range(128):
        p16_m[j % 16, j] = 1.0
    mask_bd = np.zeros((128, 4, NL), dtype=np.float32)
    for part in range(128):
        n_lo = part // 16
        for q in range(4):
            mask_bd[part, q, 8 * q + n_lo] = 1.0
    ident = np.eye(128, dtype=BF16)
    red4 = np.tile(np.eye(32, dtype=BF16), (4, 1))
    return {
        "mask_diag": np.tile(mask_diag, (N_CORES, 1)),
        "P16": np.tile(p16_m, (N_CORES, 1)),
        "mask_bd": np.tile(mask_bd, (N_CORES, 1, 1)),
        "ident": np.tile(ident, (N_CORES, 1)),
        "red4": np.tile(red4, (N_CORES, 1)),
    }


_CONSTS = None


def _get_consts():
    global _CONSTS
    if _CONSTS is None:
        _CONSTS = _consts()
    return _CONSTS


# ---------------------------------------------------------------------------
# Cached PJRT runner (jit built once per program).
# ---------------------------------------------------------------------------
_RUNNER_CACHE = {}


def _get_runner(nc):
    key = id(nc)
    if key in _RUNNER_CACHE:
        return _RUNNER_CACHE[key]

    import jax
    import jax.numpy as jnp
    from jax.sharding import Mesh, PartitionSpec, NamedSharding
    from jax.experimental.shard_map import shard_map
    from concourse.bass2jax import (
        install_neuronx_cc_hook, _bass_exec_p, partition_id_tensor)

    install_neuronx_cc_hook()

    partition_name = (nc.partition_id_tensor.name
                      if nc.partition_id_tensor else None)
    in_names, out_names, out_avals = [], [], []
    for alloc in nc.m.functions[0].allocations:
        if not isinstance(alloc, mybir.MemoryLocationSet):
            continue
        name = alloc.memorylocations[0].name
        if alloc.kind == "ExternalInput":
            if name != partition_name:
                in_names.append(name)
        elif alloc.kind == "ExternalOutput":
            out_names.append(name)
            out_avals.append(jax.core.ShapedArray(
                tuple(alloc.tensor_shape), mybir.dt.np(alloc.dtype)))
    n_params = len(in_names)
    all_in_names = list(in_names) + list(out_names)
    if partition_name is not None:
        all_in_names.append(partition_name)

    def _body(*args):
        operands = list(args)
        if partition_name is not None:
            operands.append(partition_id_tensor())
        outs = _bass_exec_p.bind(
            *operands,
            out_avals=tuple(out_avals),
            in_names=tuple(all_in_names),
            out_names=tuple(out_names),
            lowering_input_output_aliases=(),
            sim_require_finite=True,
            sim_require_nnan=True,
            nc=nc,
        )
        return tuple(outs)

    devices = jax.devices()[:N_CORES]
    mesh = Mesh(np.asarray(devices), ("core",))
    n_outs = len(out_names)
    in_specs = (PartitionSpec("core"),) * (n_params + n_outs)
    out_specs = (PartitionSpec("core"),) * n_outs
    donate = tuple(range(n_params, n_params + n_outs))
    sharded = jax.jit(
        shard_map(_body, mesh=mesh, in_specs=in_specs,
                  out_specs=out_specs, check_rep=False),
        donate_argnums=donate, keep_unused=True)
    zero_sh = tuple(NamedSharding(mesh, PartitionSpec("core"))
                    for _ in out_avals)
    make_zeros = jax.jit(
        lambda: tuple(jnp.zeros((N_CORES * av.shape[0], *av.shape[1:]),
                                av.dtype) for av in out_avals),
        out_shardings=zero_sh)

    def run(global_in: dict):
        args = [global_in[name] for name in in_names]
        out_arrs = sharded(*args, *make_zeros())
        return {name: out_arrs[i] for i, name in enumerate(out_names)}

    _RUNNER_CACHE[key] = run
    return run


_PREP_CACHE = {}


def _get_prep():
    if "prep" in _PREP_CACHE:
        return _PREP_CACHE["prep"]
    import jax
    import jax.numpy as jnp
    cpu = jax.devices("cpu")[0]
    bf = jnp.bfloat16

    def _prep(x, A, Wx, Wh, Wattn, b):
        xg = x.reshape(N * T, D).astype(bf)
        ag = A.reshape(N, H * P16N).astype(bf)
        wg = jnp.stack([Wx, Wh, Wattn], axis=1).astype(bf)
        bg = jnp.broadcast_to(b.astype(bf)[None, :], (N_CORES, 4 * H))
        return xg, ag, wg, bg

    def _post(hs):
        return hs.astype(jnp.float32)

    prep = (jax.jit(_prep, device=cpu), jax.jit(_post, device=cpu))
    _PREP_CACHE["prep"] = prep
    return prep


def kernel(**inputs):
    x = np.ascontiguousarray(np.asarray(inputs["x"], dtype=np.float32))
    A = np.ascontiguousarray(np.asarray(inputs["A"], dtype=np.float32))
    Wx = np.asarray(inputs["Wx"], dtype=np.float32)
    Wh = np.asarray(inputs["Wh"], dtype=np.float32)
    Wattn = np.asarray(inputs["Wattn"], dtype=np.float32)
    b = np.asarray(inputs["b"], dtype=np.float32)

    nc = _get_program(T)
    run = _get_runner(nc)
    prep, post = _get_prep()
    xg, ag, wgl, bg = (np.asarray(v) for v in prep(x, A, Wx, Wh, Wattn, b))
    consts = _get_consts()
    global_in = {
        "x": xg, "A": ag, "Wsh": wgl, "bias": bg,
        "mask_diag": consts["mask_diag"], "P16": consts["P16"],
        "mask_bd": consts["mask_bd"], "ident": consts["ident"],
        "red4": consts["red4"],
    }
    outs = run(global_in)
    hs = np.asarray(post(outs["hs"]))          # (256, 64, 1024) f32
    return hs.reshape(N, T, H)
